# revision 4
# baseline (speedup 1.0000x reference)
"""Trainium2 Bass kernel for nn_Network_Latent_21251498181075.

19-layer 6-wide MLP (4 residual blocks of 4 layers + 3 tail layers) over
4.19M rows, pure data parallel across 8 NeuronCores.

Per-core layout: rows are packed 21-per-column into a [126, N] SBUF tile
(21 groups x 6 features on partitions).  PE runs each layer as a
block-diagonal [126,126] matmul (float32r, full rate at N=512).  PSUM
evacuation is split across ScalarE (fused bias+leaky-relu) and VectorE
(input normalization fused into the transpose evacuation, residual adds,
and relu-only evacuations for "expansion trick" layers).  The input
transpose rides the PE (identity matmul); the output layer uses the
activation tile as the matmul *stationary* operand so its result lands in
PSUM already in natural row-major layout (no output transpose pass).

Expansion trick (exact): for a layer t in TRICK, lrelu(a) = 0.99*relu(a)
+ 0.01*a, so only relu(a) is materialized (one VectorE tensor_scalar op);
the consumer layer compensates with a second PSUM-accumulated matmul
0.01*W_{t+1}W_t applied to layer t's input, and constants fold into
biases (host-side, float64).  Block-end biases similarly fold forward so
block-end evacuations are pure residual adds.
"""

import sys

sys.path.insert(0, "/opt/trn_rl_repo")

import numpy as np

import concourse.bass as bass
from concourse import bacc
import concourse.mybir as mybir
from concourse import bass_utils
from concourse.tile import TileContext

N_CORES = 8
B_TOTAL = 4194304
B_CORE = B_TOTAL // N_CORES  # 524288
D = 6
G = 21  # packed rows per column
P_USED = G * D  # 126
COLS_CHUNK = 512  # packed columns per chunk (one PSUM bank)
ROWS_CHUNK = COLS_CHUNK * G  # 10752 rows
N_CHUNKS = 49
B_PAD = ROWS_CHUNK * N_CHUNKS  # 526848 rows per core after padding
FREE_CHUNK = ROWS_CHUNK * D // 128  # 504 natural free-dim per chunk
NL = 19
NEG_SLOPE = 0.01

TRICK = (2, 6, 10)  # relu-expansion layers (evac on VectorE)
END = (3, 7, 11, 15)  # block-end layers (residual add on VectorE)

_F32 = mybir.dt.float32
_F32R = mybir.dt.float32r


def _make_plan():
    """Static per-layer schedule shared by kernel build and host prep.

    plan[l] = (mm_list, evac); mm_list entries are (w_slot, src) with src in
    {"prev", "prev2"}; evac in {"lrelu", "relu_dve", "add_z"}.  Layer 18 is
    handled separately (stationary-operand trick).
    """
    plan = []
    slot = 0
    for l in range(18):
        mms = [(slot, "prev")]
        slot += 1
        if l - 1 in TRICK:
            mms.append((slot, "prev2"))
            slot += 1
        if l in END:
            evac = "add_z"
        elif l in TRICK:
            evac = "relu_dve"
        else:
            evac = "lrelu"
        plan.append((mms, evac))
    out_slot = slot  # layer 18 single matmul (stationary trick)
    slot += 1
    return plan, out_slot, slot


_PLAN, _OUT_SLOT, _N_SLOTS = _make_plan()


def _build_nc(repeat=1):
    nc = bacc.Bacc("TRN2", target_bir_lowering=False)
    x = nc.dram_tensor("x", [B_PAD, D], _F32, kind="ExternalInput")
    y = nc.dram_tensor("y", [B_PAD, D], _F32, kind="ExternalOutput")
    wstack = nc.dram_tensor(
        "wstack", [P_USED, _N_SLOTS * P_USED], _F32R, kind="ExternalInput"
    )
    biases = nc.dram_tensor("biases", [P_USED, NL], _F32, kind="ExternalInput")
    nrm = nc.dram_tensor("nrm", [P_USED, 2], _F32, kind="ExternalInput")
    btile = nc.dram_tensor("btile", [128, FREE_CHUNK], _F32, kind="ExternalInput")
    ident = nc.dram_tensor("ident", [128, 128], _F32, kind="ExternalInput")

    xv = x.rearrange("b d -> (b d)").rearrange(
        "(c p f) -> c p f", c=N_CHUNKS, p=128, f=FREE_CHUNK
    )
    yv = y.rearrange("b d -> (b d)").rearrange(
        "(c p f) -> c p f", c=N_CHUNKS, p=128, f=FREE_CHUNK
    )

    LRELU = mybir.ActivationFunctionType.Lrelu
    ADD = mybir.AluOpType.add
    MULT = mybir.AluOpType.mult
    MAX = mybir.AluOpType.max

    with TileContext(nc) as tc:
        with (
            tc.tile_pool(name="consts", bufs=1) as cpool,
            tc.tile_pool(name="acts", bufs=5) as apool,
            tc.tile_pool(name="zres", bufs=3) as zpool,
            tc.tile_pool(name="natio", bufs=3) as npool,
            tc.tile_pool(name="ps_t", bufs=2, space="PSUM") as tpool,
            tc.tile_pool(name="ps_l", bufs=3, space="PSUM") as lpool,
            tc.tile_pool(name="ps_o", bufs=2, space="PSUM") as opool,
        ):
            wsb = cpool.tile([P_USED, _N_SLOTS, P_USED], _F32R)
            nc.sync.dma_start(
                out=wsb[:, :, :],
                in_=wstack.rearrange("k (l m) -> k l m", l=_N_SLOTS),
            )
            bsb = cpool.tile([P_USED, NL], _F32)
            nc.sync.dma_start(out=bsb[:, :], in_=biases[:, :])
            nsb = cpool.tile([P_USED, 2], _F32)
            nc.sync.dma_start(out=nsb[:, :], in_=nrm[:, :])
            btsb = cpool.tile([128, FREE_CHUNK], _F32)
            nc.sync.dma_start(out=btsb[:, :], in_=btile[:, :])
            idsb = cpool.tile([128, 128], _F32)
            nc.sync.dma_start(out=idsb[:, :], in_=ident[:, :])

            def chunk_body(c):
                nat = npool.tile([128, FREE_CHUNK], _F32, tag="nat")
                nc.sync.dma_start(out=nat[:, :], in_=xv[c])

                # transpose chunk into packed layout: 4 identity matmuls
                tp = tpool.tile([P_USED, COLS_CHUNK], _F32, tag="tp")
                for s in range(4):
                    nc.tensor.transpose(
                        tp[:, s * 128 : (s + 1) * 128],
                        nat[:, s * P_USED : (s + 1) * P_USED],
                        idsb[:, :],
                    )

                # evacuate + normalize: x_norm = x * s + c  (per-partition)
                z = zpool.tile([128, COLS_CHUNK], _F32R, tag="z")
                nc.vector.tensor_scalar(
                    out=z[0:P_USED, :],
                    in0=tp[:, :],
                    scalar1=nsb[:, 0:1],
                    scalar2=nsb[:, 1:2],
                    op0=MULT,
                    op1=ADD,
                )

                prev, prev2 = z, None
                for l in range(18):
                    mms, evac = _PLAN[l]
                    srcs = {"prev": prev, "prev2": prev2}
                    ps = lpool.tile([P_USED, COLS_CHUNK], _F32, tag="ps")
                    for i, (slot, src) in enumerate(mms):
                        nc.tensor.matmul(
                            ps[:, :],
                            wsb[:, slot, :],
                            srcs[src][0:P_USED, :],
                            start=(i == 0),
                            stop=(i == len(mms) - 1),
                        )
                    if evac == "add_z":
                        new = zpool.tile([128, COLS_CHUNK], _F32R, tag="z")
                        nc.vector.tensor_tensor(
                            out=new[0:P_USED, :],
                            in0=ps[:, :],
                            in1=z[0:P_USED, :],
                            op=ADD,
                        )
                        z = new
                    elif evac == "relu_dve":
                        new = apool.tile([128, COLS_CHUNK], _F32R, tag="act")
                        nc.vector.tensor_scalar(
                            out=new[0:P_USED, :],
                            in0=ps[:, :],
                            scalar1=bsb[:, l : l + 1],
                            scalar2=0.0,
                            op0=ADD,
                            op1=MAX,
                        )
                    else:
                        new = apool.tile([128, COLS_CHUNK], _F32R, tag="act")
                        nc.scalar.activation(
                            out=new[0:P_USED, :],
                            in_=ps[:, :],
                            func=LRELU,
                            bias=bsb[:, l : l + 1],
                            scale=1.0,
                            alpha=NEG_SLOPE,
                        )
                    prev2, prev = prev, new

                # output layer: activation tile as stationary operand so the
                # result lands in PSUM in natural layout
                ops = opool.tile([128, FREE_CHUNK], _F32, tag="ops")
                for s in range(4):
                    nc.tensor.matmul(
                        ops[:, s * P_USED : (s + 1) * P_USED],
                        prev[0:P_USED, s * 128 : (s + 1) * 128],
                        wsb[:, _OUT_SLOT, :],
                        start=True,
                        stop=True,
                    )
                onat = npool.tile([128, FREE_CHUNK], _F32, tag="onat")
                nc.vector.tensor_tensor(
                    out=onat[:, :], in0=ops[:, :], in1=btsb[:, :], op=ADD
                )
                nc.sync.dma_start(out=yv[c], in_=onat[:, :])

            if repeat == 1:
                for c in range(N_CHUNKS):
                    chunk_body(c)
            else:
                # timing amplification: rerun the whole pass `repeat` times
                def rep_body(_i):
                    for c in range(N_CHUNKS):
                        chunk_body(c)

                tc.For_i_unrolled(0, repeat, 1, rep_body, max_unroll=1)

    nc.finalize()
    return nc


def _prep_weights(Ws, bs, bounds):
    """Host-side constant prep (float64 folding, cast to fp32).

    Mirrors _make_plan's slot order.  Maintains per-layer evac bias beta_l
    and the z-chain offset d; trick layers store relu(p + beta) and their
    consumer gets (0.99*W, prev) + (0.01*W@W_t, prev2) with constants
    folded forward.
    """
    Ws64 = Ws.astype(np.float64)
    bs64 = bs.astype(np.float64)
    lo = bounds[:, 0].astype(np.float64)
    hi = bounds[:, 1].astype(np.float64)
    scale = 1.0 / (hi - lo)
    shift = -lo * scale

    eye = np.eye(G)
    slot_mats = [None] * _N_SLOTS
    beta = np.zeros((NL, D))
    c_prev = {-1: np.zeros(D)}  # y_l = stored_l + c_l for non-trick layers
    trick_beta = {}
    d = np.zeros(D)

    for l in range(18):
        mms, evac = _PLAN[l]
        p = l - 1
        if p in TRICK:
            const = 0.01 * (Ws64[l] @ trick_beta[p])
            slot_mats[mms[0][0]] = 0.99 * Ws64[l]
            slot_mats[mms[1][0]] = 0.01 * (Ws64[l] @ Ws64[p])
        else:
            slot_mats[mms[0][0]] = Ws64[l]
            const = Ws64[l] @ c_prev[p]
        if evac == "add_z":
            d = d + bs64[l] + const
            c_prev[l] = d.copy()
        elif evac == "relu_dve":
            b = bs64[l] + const
            trick_beta[l] = b
            beta[l] = b
            c_prev[l] = None
        else:
            beta[l] = bs64[l] + const
            c_prev[l] = np.zeros(D)

    slot_mats[_OUT_SLOT] = Ws64[18]
    b18 = bs64[18] + Ws64[18] @ c_prev[17]

    def bd(mat):  # block-diagonal lhsT:  lhsT[k, m] = W[m, k]
        return np.kron(eye, mat.T)

    wstack = np.stack([bd(m) for m in slot_mats])  # [_N_SLOTS, 126, 126]
    wstack_t = np.ascontiguousarray(np.transpose(wstack, (1, 0, 2))).reshape(
        P_USED, _N_SLOTS * P_USED
    )
    biases_t = np.stack([np.tile(beta[l], G) for l in range(NL)], axis=1)
    nrm = np.stack([np.tile(scale, G), np.tile(shift, G)], axis=1)
    btile = np.broadcast_to(np.tile(b18, FREE_CHUNK // D), (128, FREE_CHUNK))

    return (
        wstack_t.astype(np.float32),
        biases_t.astype(np.float32),
        nrm.astype(np.float32),
        np.ascontiguousarray(btile).astype(np.float32),
        np.eye(128, dtype=np.float32),
    )


_NC_CACHE = {}


def kernel(X, Ws, bs, bounds):
    X = np.asarray(X, dtype=np.float32)
    Ws = np.asarray(Ws, dtype=np.float32)
    bs = np.asarray(bs, dtype=np.float32)
    bounds = np.asarray(bounds, dtype=np.float32)

    if "nc" not in _NC_CACHE:
        _NC_CACHE["nc"] = _build_nc()
    nc = _NC_CACHE["nc"]

    wstack_t, biases_t, nrm, btile, ident = _prep_weights(Ws, bs, bounds)

    pad = np.zeros((B_PAD - B_CORE, D), dtype=np.float32)
    in_maps = []
    for i in range(N_CORES):
        xc = np.concatenate([X[i * B_CORE : (i + 1) * B_CORE], pad], axis=0)
        in_maps.append(
            {
                "x": np.ascontiguousarray(xc),
                "wstack": wstack_t,
                "biases": biases_t,
                "nrm": nrm,
                "btile": btile,
                "ident": ident,
            }
        )

    res = bass_utils.run_bass_kernel_spmd(nc, in_maps, core_ids=list(range(N_CORES)))
    out = np.concatenate(
        [res.results[i]["y"][:B_CORE] for i in range(N_CORES)], axis=0
    )
    return out


# revision 7
# speedup vs baseline: 2.6713x; 2.6713x over previous
"""Trainium2 Bass kernel for nn_Network_Latent_21251498181075.

19-layer 6-wide MLP (4 residual blocks of 4 layers + 3 tail layers) over
4.19M rows, pure data parallel across 8 NeuronCores.

Per-core layout: rows are packed 21-per-column into a [126, N] SBUF tile
(21 groups x 6 features on partitions).  PE runs each layer as a
block-diagonal [126,126] matmul (float32r, full rate at N=512).  PSUM
evacuation is split across ScalarE (fused bias+leaky-relu) and VectorE
(input normalization fused into the transpose evacuation, residual adds,
and relu-only evacuations for "expansion trick" layers).  The input
transpose rides the PE (identity matmul); the output layer uses the
activation tile as the matmul *stationary* operand so its result lands in
PSUM already in natural row-major layout (no output transpose pass).

Expansion trick (exact): for a layer t in TRICK, lrelu(a) = 0.99*relu(a)
+ 0.01*a, so only relu(a) is materialized (one VectorE tensor_scalar op);
the consumer layer compensates with a second PSUM-accumulated matmul
0.01*W_{t+1}W_t applied to layer t's input, and constants fold into
biases (host-side, float64).  Block-end biases similarly fold forward so
block-end evacuations are pure residual adds.
"""

import sys

sys.path.insert(0, "/opt/trn_rl_repo")

import numpy as np

import concourse.bass as bass
from concourse import bacc
import concourse.mybir as mybir
from concourse import bass_utils
from concourse.tile import TileContext

N_CORES = 8
B_TOTAL = 4194304
B_CORE = B_TOTAL // N_CORES  # 524288
D = 6
G = 21  # packed rows per column
P_USED = G * D  # 126
COLS_CHUNK = 512  # packed columns per chunk (one PSUM bank)
ROWS_CHUNK = COLS_CHUNK * G  # 10752 rows
N_CHUNKS = 49
B_PAD = ROWS_CHUNK * N_CHUNKS  # 526848 rows per core after padding
FREE_CHUNK = ROWS_CHUNK * D // 128  # 504 natural free-dim per chunk
NL = 19
NEG_SLOPE = 0.01
INTERLEAVE = 4

TRICK = (2, 6, 10)  # relu-expansion layers (evac on VectorE)
END = (3, 7, 11, 15)  # block-end layers (residual add on VectorE)

_F32 = mybir.dt.float32
_F32R = mybir.dt.float32r


def _make_plan():
    """Static per-layer schedule shared by kernel build and host prep.

    plan[l] = (mm_list, evac); mm_list entries are (w_slot, src) with src in
    {"prev", "prev2"}; evac in {"lrelu", "relu_dve", "add_z"}.  Layer 18 is
    handled separately (stationary-operand trick).
    """
    plan = []
    slot = 0
    for l in range(18):
        mms = [(slot, "prev")]
        slot += 1
        if l - 1 in TRICK:
            mms.append((slot, "prev2"))
            slot += 1
        if l in END:
            evac = "add_z"
        elif l in TRICK:
            evac = "relu_dve"
        else:
            evac = "lrelu"
        plan.append((mms, evac))
    out_slot = slot  # layer 18 single matmul (stationary trick)
    slot += 1
    return plan, out_slot, slot


_PLAN, _OUT_SLOT, _N_SLOTS = _make_plan()


def _build_nc(repeat=1):
    nc = bacc.Bacc("TRN2", target_bir_lowering=False)
    x = nc.dram_tensor("x", [B_PAD, D], _F32, kind="ExternalInput")
    y = nc.dram_tensor("y", [B_PAD, D], _F32, kind="ExternalOutput")
    wstack = nc.dram_tensor(
        "wstack", [P_USED, _N_SLOTS * P_USED], _F32R, kind="ExternalInput"
    )
    biases = nc.dram_tensor("biases", [P_USED, NL], _F32, kind="ExternalInput")
    nrm = nc.dram_tensor("nrm", [P_USED, 2], _F32, kind="ExternalInput")
    btile = nc.dram_tensor("btile", [128, FREE_CHUNK], _F32, kind="ExternalInput")
    ident = nc.dram_tensor("ident", [128, 128], _F32, kind="ExternalInput")

    xv = x.rearrange("b d -> (b d)").rearrange(
        "(c p f) -> c p f", c=N_CHUNKS, p=128, f=FREE_CHUNK
    )
    yv = y.rearrange("b d -> (b d)").rearrange(
        "(c p f) -> c p f", c=N_CHUNKS, p=128, f=FREE_CHUNK
    )

    LRELU = mybir.ActivationFunctionType.Lrelu
    ADD = mybir.AluOpType.add
    MULT = mybir.AluOpType.mult
    MAX = mybir.AluOpType.max

    # Interleave INTER chunks through the layer loop so PE/ACT/DVE pipeline
    # across chunks instead of serializing on each chunk's matmul->evac chain.
    INTER = INTERLEAVE

    with TileContext(nc) as tc:
        with (
            tc.tile_pool(name="consts", bufs=1) as cpool,
            tc.tile_pool(name="acts", bufs=3 * INTER) as apool,
            tc.tile_pool(name="zres", bufs=2 * INTER) as zpool,
            tc.tile_pool(name="natio", bufs=2 * INTER) as npool,
            tc.tile_pool(name="ps_l", bufs=6, space="PSUM") as lpool,
            tc.tile_pool(name="ps_o", bufs=2, space="PSUM") as opool,
        ):
            tpool = lpool  # transposes share the layer-psum bank pool
            wsb = cpool.tile([P_USED, _N_SLOTS, P_USED], _F32R)
            nc.sync.dma_start(
                out=wsb[:, :, :],
                in_=wstack.rearrange("k (l m) -> k l m", l=_N_SLOTS),
            )
            bsb = cpool.tile([P_USED, NL], _F32)
            nc.sync.dma_start(out=bsb[:, :], in_=biases[:, :])
            nsb = cpool.tile([P_USED, 2], _F32)
            nc.sync.dma_start(out=nsb[:, :], in_=nrm[:, :])
            btsb = cpool.tile([128, FREE_CHUNK], _F32)
            nc.sync.dma_start(out=btsb[:, :], in_=btile[:, :])
            idsb = cpool.tile([128, 128], _F32)
            nc.sync.dma_start(out=idsb[:, :], in_=ident[:, :])

            def start_chunk(c):
                nat = npool.tile([128, FREE_CHUNK], _F32, tag="nat")
                nc.sync.dma_start(out=nat[:, :], in_=xv[c])

                # transpose chunk into packed layout: 4 identity matmuls
                tp = tpool.tile([P_USED, COLS_CHUNK], _F32, tag="ps")
                for s in range(4):
                    nc.tensor.transpose(
                        tp[:, s * 128 : (s + 1) * 128],
                        nat[:, s * P_USED : (s + 1) * P_USED],
                        idsb[:, :],
                    )

                # evacuate + normalize: x_norm = x * s + c  (per-partition)
                z = zpool.tile([128, COLS_CHUNK], _F32R, tag="z")
                nc.vector.tensor_scalar(
                    out=z[0:P_USED, :],
                    in0=tp[:, :],
                    scalar1=nsb[:, 0:1],
                    scalar2=nsb[:, 1:2],
                    op0=MULT,
                    op1=ADD,
                )
                return {"z": z, "prev": z, "prev2": None, "c": c}

            def layer_step(st, l):
                mms, evac = _PLAN[l]
                srcs = {"prev": st["prev"], "prev2": st["prev2"]}
                ps = lpool.tile([P_USED, COLS_CHUNK], _F32, tag="ps")
                for i, (slot, src) in enumerate(mms):
                    nc.tensor.matmul(
                        ps[:, :],
                        wsb[:, slot, :],
                        srcs[src][0:P_USED, :],
                        start=(i == 0),
                        stop=(i == len(mms) - 1),
                    )
                if evac == "add_z":
                    new = zpool.tile([128, COLS_CHUNK], _F32R, tag="z")
                    nc.vector.tensor_tensor(
                        out=new[0:P_USED, :],
                        in0=ps[:, :],
                        in1=st["z"][0:P_USED, :],
                        op=ADD,
                    )
                    st["z"] = new
                elif evac == "relu_dve":
                    new = apool.tile([128, COLS_CHUNK], _F32R, tag="act")
                    nc.vector.tensor_scalar(
                        out=new[0:P_USED, :],
                        in0=ps[:, :],
                        scalar1=bsb[:, l : l + 1],
                        scalar2=0.0,
                        op0=ADD,
                        op1=MAX,
                    )
                else:
                    new = apool.tile([128, COLS_CHUNK], _F32R, tag="act")
                    nc.scalar.activation(
                        out=new[0:P_USED, :],
                        in_=ps[:, :],
                        func=LRELU,
                        bias=bsb[:, l : l + 1],
                        scale=1.0,
                        alpha=NEG_SLOPE,
                    )
                st["prev2"], st["prev"] = st["prev"], new

            def out_step(st):
                # output layer: activation tile as stationary operand so the
                # result lands in PSUM in natural layout
                prev = st["prev"]
                ops = opool.tile([128, FREE_CHUNK], _F32, tag="ops")
                for s in range(4):
                    nc.tensor.matmul(
                        ops[:, s * P_USED : (s + 1) * P_USED],
                        prev[0:P_USED, s * 128 : (s + 1) * 128],
                        wsb[:, _OUT_SLOT, :],
                        start=True,
                        stop=True,
                    )
                onat = npool.tile([128, FREE_CHUNK], _F32, tag="onat")
                nc.vector.tensor_tensor(
                    out=onat[:, :], in0=ops[:, :], in1=btsb[:, :], op=ADD
                )
                nc.sync.dma_start(out=yv[st["c"]], in_=onat[:, :])

            def full_pass():
                for g in range(0, N_CHUNKS, INTER):
                    cs = list(range(g, min(g + INTER, N_CHUNKS)))
                    states = [start_chunk(c) for c in cs]
                    for l in range(18):
                        for st in states:
                            layer_step(st, l)
                    for st in states:
                        out_step(st)

            if repeat == 1:
                full_pass()
            else:
                # timing amplification: rerun the whole pass `repeat` times
                tc.For_i_unrolled(0, repeat, 1, lambda _i: full_pass(), max_unroll=1)

    nc.finalize()
    return nc


def _prep_weights(Ws, bs, bounds):
    """Host-side constant prep (float64 folding, cast to fp32).

    Mirrors _make_plan's slot order.  Maintains per-layer evac bias beta_l
    and the z-chain offset d; trick layers store relu(p + beta) and their
    consumer gets (0.99*W, prev) + (0.01*W@W_t, prev2) with constants
    folded forward.
    """
    Ws64 = Ws.astype(np.float64)
    bs64 = bs.astype(np.float64)
    lo = bounds[:, 0].astype(np.float64)
    hi = bounds[:, 1].astype(np.float64)
    scale = 1.0 / (hi - lo)
    shift = -lo * scale

    eye = np.eye(G)
    slot_mats = [None] * _N_SLOTS
    beta = np.zeros((NL, D))
    c_prev = {-1: np.zeros(D)}  # y_l = stored_l + c_l for non-trick layers
    trick_beta = {}
    d = np.zeros(D)

    for l in range(18):
        mms, evac = _PLAN[l]
        p = l - 1
        if p in TRICK:
            const = 0.01 * (Ws64[l] @ trick_beta[p])
            slot_mats[mms[0][0]] = 0.99 * Ws64[l]
            slot_mats[mms[1][0]] = 0.01 * (Ws64[l] @ Ws64[p])
        else:
            slot_mats[mms[0][0]] = Ws64[l]
            const = Ws64[l] @ c_prev[p]
        if evac == "add_z":
            d = d + bs64[l] + const
            c_prev[l] = d.copy()
        elif evac == "relu_dve":
            b = bs64[l] + const
            trick_beta[l] = b
            beta[l] = b
            c_prev[l] = None
        else:
            beta[l] = bs64[l] + const
            c_prev[l] = np.zeros(D)

    slot_mats[_OUT_SLOT] = Ws64[18]
    b18 = bs64[18] + Ws64[18] @ c_prev[17]

    def bd(mat):  # block-diagonal lhsT:  lhsT[k, m] = W[m, k]
        return np.kron(eye, mat.T)

    wstack = np.stack([bd(m) for m in slot_mats])  # [_N_SLOTS, 126, 126]
    wstack_t = np.ascontiguousarray(np.transpose(wstack, (1, 0, 2))).reshape(
        P_USED, _N_SLOTS * P_USED
    )
    biases_t = np.stack([np.tile(beta[l], G) for l in range(NL)], axis=1)
    nrm = np.stack([np.tile(scale, G), np.tile(shift, G)], axis=1)
    btile = np.broadcast_to(np.tile(b18, FREE_CHUNK // D), (128, FREE_CHUNK))

    return (
        wstack_t.astype(np.float32),
        biases_t.astype(np.float32),
        nrm.astype(np.float32),
        np.ascontiguousarray(btile).astype(np.float32),
        np.eye(128, dtype=np.float32),
    )


_NC_CACHE = {}


def kernel(X, Ws, bs, bounds):
    X = np.asarray(X, dtype=np.float32)
    Ws = np.asarray(Ws, dtype=np.float32)
    bs = np.asarray(bs, dtype=np.float32)
    bounds = np.asarray(bounds, dtype=np.float32)

    if "nc" not in _NC_CACHE:
        _NC_CACHE["nc"] = _build_nc()
    nc = _NC_CACHE["nc"]

    wstack_t, biases_t, nrm, btile, ident = _prep_weights(Ws, bs, bounds)

    pad = np.zeros((B_PAD - B_CORE, D), dtype=np.float32)
    in_maps = []
    for i in range(N_CORES):
        xc = np.concatenate([X[i * B_CORE : (i + 1) * B_CORE], pad], axis=0)
        in_maps.append(
            {
                "x": np.ascontiguousarray(xc),
                "wstack": wstack_t,
                "biases": biases_t,
                "nrm": nrm,
                "btile": btile,
                "ident": ident,
            }
        )

    res = bass_utils.run_bass_kernel_spmd(nc, in_maps, core_ids=list(range(N_CORES)))
    out = np.concatenate(
        [res.results[i]["y"][:B_CORE] for i in range(N_CORES)], axis=0
    )
    return out
